# revision 1
# baseline (speedup 1.0000x reference)
"""Trainium2 Bass kernel for nn_CorrelatedAttentionBlock_81286551044296.

Shapes (hardcoded): x (4, 256, 512, 64) f32; Wq/Wk/Wv/Wo (256,256); b* (256,);
log_tau (1,).  8 NeuronCores, sharded over (batch b, F-half): core = b*2 + fh,
each core handles x[b, :, :, fh*32:(fh+1)*32] -> out same slice.  Fully
independent shards (projections are pointwise over (t,f), the time-normalization
and Dh x Dh covariance are per (h,f) and contract only over T) -> no collectives.

Algorithm per (core, f):  let X = x[b,:,:,f]  (C=256 x T=512)
  G    = X X^T                  (Gram over time, 256x256)         [PE]
  A|Ak = G @ [Wq^T | Wk^T]                                        [PE]
  ssq[e] = sum_i Wq^T[i,e] A[i,e]  = ||Q[:,e]||^2   (ones-matmul) [DVE TT + PE]
  ssk[d] likewise from Ak
  S2[d,e] = ssk[d]*ssq[e]   (K=1 outer-product matmul)            [PE]
  scale = sqrt(inv_tau^2 / S2)                                    [DVE recip + ACT sqrt]
  cov[d,e] = (Wk G Wq^T)[d,e]  (2 diag 128-blocks only)           [PE]
  att = softmax over 32-wide head segments:
        exp(cov*scale + addmask) with addmask=-30 off-segment     [DVE TT, ACT exp+accum]
  U^T[e,i] = sum_d att[d,e] Wv[d,i]                               [PE]
  Z[i,c]   = sum_e U[i,e] Wo^T[e,c]                               [PE]
  out^T[c,t] = sum_i Z[i,c] X[i,t]  + bo                          [PE + ACT]

Matmul operands in bf16 (fp32 PSUM accumulation); normalization/softmax scalars
in fp32.  Host pre-transposes x into time-major (for G) and channel-major (for
the final projection) bf16 layouts.  Biases bq/bk/bv are zero for this problem
instance; a numpy fallback handles the general case.
"""

import numpy as np
import ml_dtypes

B, C, T, FQ = 4, 256, 512, 64
H, DH = 8, 32
FL = FQ // 2  # 32 f per core
N_CORES = 8

BF16 = ml_dtypes.bfloat16

_PROGRAM_CACHE = {}


def _build_program(inv_tau_sq: float, reps: int = 1):
    import concourse.bacc as bacc
    import concourse.tile as tile
    from concourse import mybir
    from contextlib import ExitStack

    dt = mybir.dt
    AF = mybir.ActivationFunctionType

    nc = bacc.Bacc()
    # device inputs (per core)
    x_tm = nc.declare_dram_parameter("x_tm", [FL, 128, 4, 256], dt.bfloat16, isOutput=False)
    x_cm = nc.declare_dram_parameter("x_cm", [FL, 128, 2, 512], dt.bfloat16, isOutput=False)
    wqk = nc.declare_dram_parameter("wqk", [128, 2, 512], dt.bfloat16, isOutput=False)
    wv = nc.declare_dram_parameter("wv", [128, 2, 256], dt.bfloat16, isOutput=False)
    wo = nc.declare_dram_parameter("wo", [128, 2, 256], dt.bfloat16, isOutput=False)
    amask = nc.declare_dram_parameter("amask", [128, 256], dt.float32, isOutput=False)
    bo_c = nc.declare_dram_parameter("bo_c", [128, 2], dt.float32, isOutput=False)
    ones_c = nc.declare_dram_parameter("ones_c", [128, 1], dt.bfloat16, isOutput=False)
    out_d = nc.declare_dram_parameter("out_d", [FL, 128, 2, 512], dt.float32, isOutput=True)

    with tile.TileContext(nc) as tc:
        with ExitStack() as ctx:
            wpool = ctx.enter_context(tc.tile_pool(name="w", bufs=1))
            xpool = ctx.enter_context(tc.tile_pool(name="x", bufs=4))
            spool = ctx.enter_context(tc.tile_pool(name="s", bufs=4))
            opool = ctx.enter_context(tc.tile_pool(name="o", bufs=4))
            ppool = ctx.enter_context(tc.tile_pool(name="ps", bufs=8, space="PSUM"))

            wqk_t = wpool.tile([128, 2, 512], dt.bfloat16)
            nc.sync.dma_start(wqk_t[:], wqk[:])
            wv_t = wpool.tile([128, 2, 256], dt.bfloat16)
            nc.sync.dma_start(wv_t[:], wv[:])
            wo_t = wpool.tile([128, 2, 256], dt.bfloat16)
            nc.sync.dma_start(wo_t[:], wo[:])
            am_t = wpool.tile([128, 256], dt.float32)
            nc.sync.dma_start(am_t[:], amask[:])
            bo_t = wpool.tile([128, 2], dt.float32)
            nc.sync.dma_start(bo_t[:], bo_c[:])
            on_t = wpool.tile([128, 1], dt.bfloat16)
            nc.sync.dma_start(on_t[:], ones_c[:])

            for f_iter in range(FL * reps):
                f = f_iter % FL  # reps>1 only for timing amplification
                xtm = xpool.tile([128, 4, 256], dt.bfloat16, tag="xtm")
                nc.sync.dma_start(xtm[:], x_tm[f])
                xcm = xpool.tile([128, 2, 512], dt.bfloat16, tag="xcm")
                nc.sync.dma_start(xcm[:], x_cm[f])

                # G[i,j] = sum_t X[i,t] X[j,t]
                g_ps = ppool.tile([128, 2, 256], dt.float32, tag="ps1")
                for ib in range(2):
                    for tci in range(4):
                        nc.tensor.matmul(
                            g_ps[:, ib, :],
                            lhsT=xtm[:, tci, ib * 128:(ib + 1) * 128],
                            rhs=xtm[:, tci, :],
                            start=(tci == 0), stop=(tci == 3),
                        )
                g_bf = spool.tile([128, 2, 256], dt.bfloat16, tag="gbf")
                nc.scalar.copy(g_bf[:], g_ps[:])

                # A|Ak = G @ [WqT | WkT]
                aak_bf = spool.tile([128, 2, 512], dt.bfloat16, tag="aakbf")
                for ic in range(2):
                    aak_ps = ppool.tile([128, 512], dt.float32, tag="ps1")
                    for jc in range(2):
                        nc.tensor.matmul(
                            aak_ps[:],
                            lhsT=g_bf[:, jc, ic * 128:(ic + 1) * 128],
                            rhs=wqk_t[:, jc, :],
                            start=(jc == 0), stop=(jc == 1),
                        )
                    if ic == 0:
                        nc.vector.tensor_copy(aak_bf[:, ic, :], aak_ps[:])
                    else:
                        nc.scalar.copy(aak_bf[:, ic, :], aak_ps[:])

                # prod = [WqT*A | WkT*Ak]; column sums give ssq|ssk
                prod = spool.tile([128, 2, 512], dt.bfloat16, tag="prod")
                nc.vector.tensor_mul(prod[:], wqk_t[:], aak_bf[:])
                ss_ps = ppool.tile([128, 512], dt.float32, tag="ps1")
                for ic in range(2):
                    nc.tensor.matmul(
                        ss_ps[0:1, :], lhsT=on_t[:], rhs=prod[:, ic, :],
                        start=(ic == 0), stop=(ic == 1),
                    )
                rows = spool.tile([1, 512], dt.bfloat16, tag="rows")
                nc.scalar.copy(rows[:], ss_ps[0:1, :])

                # S2[d,e] = ssk[d] * ssq[e]  (two 128-blocks)
                s2_ps = ppool.tile([128, 256], dt.float32, tag="ps1")
                for b2 in range(2):
                    nc.tensor.matmul(
                        s2_ps[:, b2 * 128:(b2 + 1) * 128],
                        lhsT=rows[0:1, 256 + b2 * 128: 256 + (b2 + 1) * 128],
                        rhs=rows[0:1, b2 * 128:(b2 + 1) * 128],
                        start=True, stop=True,
                    )
                s2r = spool.tile([128, 256], dt.float32, tag="s2r")
                nc.vector.reciprocal(s2r[:], s2_ps[:])
                scl = spool.tile([128, 256], dt.float32, tag="scl")
                nc.scalar.activation(scl[:], s2r[:], AF.Sqrt, scale=float(inv_tau_sq))

                # cov diag blocks: cov[d,e] = (Wk G WqT)[d,e]
                cov_ps = ppool.tile([128, 256], dt.float32, tag="ps1")
                for b2 in range(2):
                    for ic in range(2):
                        nc.tensor.matmul(
                            cov_ps[:, b2 * 128:(b2 + 1) * 128],
                            lhsT=wqk_t[:, ic, 256 + b2 * 128: 256 + (b2 + 1) * 128],
                            rhs=aak_bf[:, ic, b2 * 128:(b2 + 1) * 128],
                            start=(ic == 0), stop=(ic == 1),
                        )

                covt = spool.tile([128, 256], dt.float32, tag="covt")
                nc.vector.tensor_mul(covt[:], cov_ps[:], scl[:])
                covm = spool.tile([128, 256], dt.float32, tag="covm")
                nc.vector.tensor_add(covm[:], covt[:], am_t[:])

                e_bf = spool.tile([128, 256], dt.bfloat16, tag="ebf")
                rsum = spool.tile([128, 2], dt.float32, tag="rsum")
                for b2 in range(2):
                    nc.scalar.activation(
                        e_bf[:, b2 * 128:(b2 + 1) * 128],
                        covm[:, b2 * 128:(b2 + 1) * 128],
                        AF.Exp, accum_out=rsum[:, b2:b2 + 1],
                    )
                rinv = spool.tile([128, 2], dt.float32, tag="rinv")
                nc.vector.reciprocal(rinv[:], rsum[:])
                att = spool.tile([128, 256], dt.bfloat16, tag="att")
                for b2 in range(2):
                    nc.vector.tensor_scalar_mul(
                        att[:, b2 * 128:(b2 + 1) * 128],
                        e_bf[:, b2 * 128:(b2 + 1) * 128],
                        rinv[:, b2:b2 + 1],
                    )

                # U^T[e,i] = sum_d att[d,e] Wv[d,i]
                ut_ps = ppool.tile([128, 2, 256], dt.float32, tag="ps1")
                for b2 in range(2):
                    nc.tensor.matmul(
                        ut_ps[:, b2, :],
                        lhsT=att[:, b2 * 128:(b2 + 1) * 128],
                        rhs=wv_t[:, b2, :],
                        start=True, stop=True,
                    )
                ut_bf = spool.tile([128, 2, 256], dt.bfloat16, tag="utbf")
                nc.scalar.copy(ut_bf[:], ut_ps[:])

                # Z[i,c] = sum_e U[i,e] WoT[e,c]
                z_ps = ppool.tile([128, 2, 256], dt.float32, tag="ps1")
                for ib in range(2):
                    for b2 in range(2):
                        nc.tensor.matmul(
                            z_ps[:, ib, :],
                            lhsT=ut_bf[:, b2, ib * 128:(ib + 1) * 128],
                            rhs=wo_t[:, b2, :],
                            start=(b2 == 0), stop=(b2 == 1),
                        )
                z_bf = spool.tile([128, 2, 256], dt.bfloat16, tag="zbf")
                nc.vector.tensor_copy(z_bf[:], z_ps[:])

                # out^T[c,t] = sum_i Z[i,c] X[i,t] + bo
                fout = opool.tile([128, 2, 512], dt.float32, tag="fout")
                for cb in range(2):
                    o_ps = ppool.tile([128, 512], dt.float32, tag="ps1")
                    for ib in range(2):
                        nc.tensor.matmul(
                            o_ps[:],
                            lhsT=z_bf[:, ib, cb * 128:(cb + 1) * 128],
                            rhs=xcm[:, ib, :],
                            start=(ib == 0), stop=(ib == 1),
                        )
                    nc.scalar.activation(
                        fout[:, cb, :], o_ps[:], AF.Identity,
                        bias=bo_t[:, cb:cb + 1],
                    )
                nc.sync.dma_start(out_d[f], fout[:])

    nc.finalize()
    return nc


def get_program(inv_tau_sq: float):
    key = round(float(inv_tau_sq), 12)
    if key not in _PROGRAM_CACHE:
        _PROGRAM_CACHE[key] = _build_program(inv_tau_sq)
    return _PROGRAM_CACHE[key]


def make_core_inputs(x, Wq, Wk, Wv, Wo, bo):
    """Host-side shard + layout prep. Returns list of 8 in_maps."""
    wqk = np.concatenate([Wq.T, Wk.T], axis=1).astype(BF16)  # (256, 512)
    wqk = wqk.reshape(2, 128, 512).transpose(1, 0, 2).copy()
    wv = Wv.astype(BF16).reshape(2, 128, 256).transpose(1, 0, 2).copy()  # [d,i] rows d
    wo = Wo.T.astype(BF16).reshape(2, 128, 256).transpose(1, 0, 2).copy()
    # additive mask: 0 on same 32-head segment, -30 off
    p = np.arange(128)[:, None] // 32
    e = np.arange(128)[None, :] // 32
    blk = np.where(p == e, 0.0, -30.0).astype(np.float32)
    amask = np.concatenate([blk, blk], axis=1).copy()  # [128, 256]
    bo_c = bo.astype(np.float32).reshape(2, 128).T.copy()
    ones_c = np.ones((128, 1), BF16)

    in_maps = []
    for core in range(N_CORES):
        b, fh = divmod(core, 2)
        xs = x[b, :, :, fh * FL:(fh + 1) * FL]  # (C, T, FL)
        xtm = np.ascontiguousarray(xs.transpose(2, 1, 0)).astype(BF16)  # (FL, T, C)
        xtm = xtm.reshape(FL, 4, 128, 256).transpose(0, 2, 1, 3).copy()
        xcm = np.ascontiguousarray(xs.transpose(2, 0, 1)).astype(BF16)  # (FL, C, T)
        xcm = xcm.reshape(FL, 2, 128, 512).transpose(0, 2, 1, 3).copy()
        in_maps.append({
            "x_tm": xtm, "x_cm": xcm, "wqk": wqk, "wv": wv, "wo": wo,
            "amask": amask, "bo_c": bo_c, "ones_c": ones_c,
        })
    return in_maps


def assemble_output(results):
    out = np.empty((B, C, T, FQ), dtype=np.float32)
    for core in range(N_CORES):
        b, fh = divmod(core, 2)
        od = results[core]["out_d"]  # (FL, 128, 2, 512)
        # od[f, p, cb, t] = out[b, cb*128+p, t, f0+f]
        o = od.transpose(2, 1, 3, 0).reshape(C, T, FL)
        out[b, :, :, fh * FL:(fh + 1) * FL] = o
    return out


def _numpy_reference(x, Wq, bq, Wk, bk, Wv, bv, Wo, bo, log_tau):
    xf = np.transpose(x, (0, 2, 3, 1)).astype(np.float64)

    def split(t):
        return np.transpose(t.reshape(B, T, FQ, H, DH), (0, 3, 1, 2, 4))

    Qh = split(xf @ Wq.T.astype(np.float64) + bq)
    Kh = split(xf @ Wk.T.astype(np.float64) + bk)
    Vh = split(xf @ Wv.T.astype(np.float64) + bv)

    def l2t(a, eps=1e-8):
        return a / np.sqrt(np.clip(np.sum(a * a, axis=2, keepdims=True), eps, None))

    Qh, Kh = l2t(Qh), l2t(Kh)
    tau = np.clip(np.exp(log_tau.astype(np.float64)), 1e-4, 10.0)
    cov = np.einsum('bhtfd,bhtfe->bhfde', Kh, Qh) / tau
    cov = cov - cov.max(axis=-1, keepdims=True)
    ecov = np.exp(cov)
    att = ecov / ecov.sum(axis=-1, keepdims=True)
    out_h = np.einsum('bhtfd,bhfde->bhtfe', Vh, att)
    out_tf = np.transpose(out_h, (0, 2, 3, 1, 4)).reshape(B, T, FQ, C)
    out_tf = out_tf @ Wo.T.astype(np.float64) + bo
    return np.transpose(out_tf, (0, 3, 1, 2)).astype(np.float32)


def kernel(x, Wq, bq, Wk, bk, Wv, bv, Wo, bo, log_tau):
    x = np.asarray(x, dtype=np.float32)
    Wq, Wk, Wv, Wo = (np.asarray(w, dtype=np.float32) for w in (Wq, Wk, Wv, Wo))
    bq, bk, bv, bo = (np.asarray(v, dtype=np.float32) for v in (bq, bk, bv, bo))
    log_tau = np.asarray(log_tau, dtype=np.float32)

    if np.any(bq) or np.any(bk) or np.any(bv):
        # general-case fallback (never hit for this problem's inputs)
        return _numpy_reference(x, Wq, bq, Wk, bk, Wv, bv, Wo, bo, log_tau)

    from concourse.bass_utils import run_bass_kernel_spmd

    tau = float(np.clip(np.exp(log_tau[0]), 1e-4, 10.0))
    inv_tau_sq = 1.0 / (tau * tau)
    nc = get_program(inv_tau_sq)
    in_maps = make_core_inputs(x, Wq, Wk, Wv, Wo, bo)
    res = run_bass_kernel_spmd(nc, in_maps, list(range(N_CORES)))
    return assemble_output(res.results)

